# revision 4
# baseline (speedup 1.0000x reference)
"""VQ codebook kernel for Trainium2, data-parallel over 8 NeuronCores.

Problem (hardcoded shapes): z (16, 512, 64, 64) f32, codebook (1024, 512) f32.
Returns (z_q, q_loss, distance_prob) matching the reference:
    z_flat = z.transpose(0,2,3,1).reshape(-1, 512)          # (65536, 512)
    dist   = ||z||^2 + ||c||^2 - 2 z @ c.T                  # (65536, 1024)
    min_idx = argmin(dist, axis=1)  (first index on fp32 ties)
    distance_prob = softmax(-dist / 0.5, axis=1)
    z_q = codebook[min_idx]  -> reshaped back to (16, 512, 64, 64)
    q_loss = 1.25 * mean((z_q - z_flat)^2)

Sharding: data-parallel on z rows — core i handles b in {2i, 2i+1} (8192 rows),
codebook replicated. Scalar loss reduced on host.

Numerics: the argmin must replicate the reference's fp32 rounding of
dist = fl(fl(zn + cn) - 2M) (values ~512, ulp 6.1e-5) including first-index
tie-breaks, so the matmul uses fp32 on the PE (error ~1e-8) and the kernel
computes negdist = fl(2M - t) (= exactly -dist) whose first-index max is the
argmin.  Row norms zn may be computed in any fp32 order: a per-row constant
shift moves the whole row rigidly on the fp32 grid within a binade, so the
quantized comparison structure is preserved.
"""

import sys

for _p in ("/opt/trn_rl_repo", "/root/.axon_site/_ro/trn_rl_repo"):
    if _p not in sys.path:
        sys.path.append(_p)

import numpy as np

import concourse.bass as bass
import concourse.tile as tile
from concourse import mybir
from concourse.bass_utils import run_bass_kernel_spmd
from concourse.masks import make_identity

P = 128
D = 512
K = 1024
B, H, W = 16, 64, 64
N_CORES = 8
B_PER_CORE = B // N_CORES            # 2
ROWS_PER_CORE = B_PER_CORE * H * W   # 8192
N_SUB = ROWS_PER_CORE // P           # 64 subtiles of 128 rows
SUB_PER_BLK = 4                      # 512 rows per z-load block
N_BLK = N_SUB // SUB_PER_BLK         # 16
SUB_PER_B = (H * W) // P             # 32 subtiles per b index

F32 = mybir.dt.float32
U32 = mybir.dt.uint32


def _split_waits(nc, limit=1):
    """This walrus build accepts at most one sync-wait per instruction; move
    extra waits onto preceding same-engine NOPs."""
    for fn in nc.m.functions:
        for blk in fn.blocks:
            new_insts = []
            for inst in blk.instructions:
                si = inst.sync_info
                if si is not None and si.on_wait and len(si.on_wait) > limit:
                    waits = list(si.on_wait)
                    extra, keep = waits[:-limit], waits[-limit:]
                    while extra:
                        chunk, extra = extra[:limit], extra[limit:]
                        nop = mybir.InstNoOp(
                            name=nc.get_next_instruction_name(),
                            engine=inst.engine,
                            ins=[],
                            outs=[],
                            sync_info=mybir.SyncInfo(on_wait=chunk, on_update=[]),
                        )
                        nc.register_instruction(nop)
                        new_insts.append(nop)
                    inst.sync_info = mybir.SyncInfo(
                        on_wait=keep, on_update=list(si.on_update or [])
                    )
                new_insts.append(inst)
            blk.instructions[:] = new_insts


def _build():
    nc = bass.Bass()

    zc = nc.declare_dram_parameter("zc", [B_PER_CORE, D, H * W], F32, isOutput=False)
    cbt2 = nc.declare_dram_parameter("cbt2", [D, K], F32, isOutput=False)  # 2*cb.T
    cnp = nc.declare_dram_parameter("cn", [1, K], F32, isOutput=False)
    znr = nc.declare_dram_parameter("znr", [P, N_SUB], F32, isOutput=False)
    cbg = nc.declare_dram_parameter("cbg", [K, D], F32, isOutput=False)  # gather table

    probs_o = nc.declare_dram_parameter("probs", [ROWS_PER_CORE, K], F32, isOutput=True)
    zq_o = nc.declare_dram_parameter("zq", [B_PER_CORE, D, H * W], F32, isOutput=True)
    m_o = nc.declare_dram_parameter("mrow", [P, N_SUB], F32, isOutput=True)
    idx_o = nc.declare_dram_parameter("idxr", [P, N_SUB], U32, isOutput=True)

    with tile.TileContext(nc) as tc:
        with (
            tc.tile_pool(name="statics", bufs=1) as statics,
            tc.tile_pool(name="zpool", bufs=3) as zpool,
            tc.tile_pool(name="tpool", bufs=3) as tpool,
            tc.tile_pool(name="ndpool", bufs=3) as ndpool,
            tc.tile_pool(name="epool", bufs=3) as epool,
            tc.tile_pool(name="prpool", bufs=3) as prpool,
            tc.tile_pool(name="gqpool", bufs=3) as gqpool,
            tc.tile_pool(name="zqpool", bufs=6) as zqpool,
            tc.tile_pool(name="small", bufs=8) as small,
            tc.tile_pool(name="pspool", bufs=2, space="PSUM") as pspool,
            tc.tile_pool(name="tppool", bufs=4, space="PSUM") as tppool,
        ):
            # resident tensors
            cbt = statics.tile([P, D // P, K], F32)       # 2*cb.T as [di, do, k]
            nc.sync.dma_start(
                out=cbt[:],
                in_=cbt2[:].rearrange("(do di) k -> di do k", di=P),
            )
            cn_sb = statics.tile([P, K], F32)
            nc.sync.dma_start(out=cn_sb[:], in_=cnp[:].to_broadcast([P, K]))
            znr_sb = statics.tile([P, N_SUB], F32)
            nc.sync.dma_start(out=znr_sb[:], in_=znr[:])
            ident = statics.tile([P, P], F32)
            make_identity(nc, ident[:])
            msb = statics.tile([P, N_SUB], F32)
            isb = statics.tile([P, N_SUB], U32)

            for blk in range(N_BLK):
                bb = blk // (N_BLK // B_PER_CORE)
                poff = (blk * SUB_PER_BLK * P) % (H * W)
                zt = zpool.tile([P, D // P, SUB_PER_BLK * P], F32)
                nc.sync.dma_start(
                    out=zt[:],
                    in_=zc[bb].rearrange("(do di) x -> di do x", di=P)[
                        :, :, poff : poff + SUB_PER_BLK * P
                    ],
                )
                for sub in range(SUB_PER_BLK):
                    st = blk * SUB_PER_BLK + sub
                    ps = pspool.tile([P, K], F32)
                    for n in range(K // 512):
                        for k in range(D // P):
                            nc.tensor.matmul(
                                ps[:, n * 512 : (n + 1) * 512],
                                lhsT=zt[:, k, sub * P : (sub + 1) * P],
                                rhs=cbt[:, k, n * 512 : (n + 1) * 512],
                                start=(k == 0),
                                stop=(k == D // P - 1),
                            )
                    # t = fl(cn + zn)  (bit-exact fp32 add on ACT)
                    t = tpool.tile([P, K], F32)
                    nc.scalar.activation(
                        t[:], cn_sb[:], mybir.ActivationFunctionType.Identity,
                        bias=znr_sb[:, st : st + 1], scale=1.0,
                    )
                    # negdist = fl(2M - t) = -dist
                    nd = ndpool.tile([P, K], F32)
                    nc.vector.tensor_tensor(
                        nd[:], ps[:], t[:], mybir.AluOpType.subtract
                    )
                    # m8/idx8: first-index argmax of negdist == argmin of dist
                    m8 = small.tile([P, 8], F32)
                    nc.vector.max(m8[:], nd[:])
                    idx8 = small.tile([P, 8], U32)
                    nc.vector.max_index(idx8[:], m8[:], nd[:])
                    # e = exp(2*negdist - 2*m), rowsum via accum
                    biasm = small.tile([P, 1], F32)
                    nc.scalar.mul(biasm[:], m8[:, :1], -2.0)
                    e = epool.tile([P, K], F32)
                    s = small.tile([P, 1], F32)
                    nc.scalar.activation(
                        e[:], nd[:], mybir.ActivationFunctionType.Exp,
                        bias=biasm[:], scale=2.0, accum_out=s[:],
                    )
                    r = small.tile([P, 1], F32)
                    nc.vector.reciprocal(r[:], s[:])
                    pr = prpool.tile([P, K], F32)
                    nc.scalar.mul(pr[:], e[:], r[:])
                    nc.sync.dma_start(
                        out=probs_o[st * P : (st + 1) * P, :], in_=pr[:]
                    )
                    # z_q gather + transpose to [d, pix]
                    gq = gqpool.tile([P, D], F32)
                    nc.gpsimd.indirect_dma_start(
                        out=gq[:], out_offset=None,
                        in_=cbg[:],
                        in_offset=bass.IndirectOffsetOnAxis(ap=idx8[:, :1], axis=0),
                    )
                    for k in range(D // P):
                        pst = tppool.tile([P, P], F32)
                        nc.tensor.transpose(
                            pst[:], gq[:, k * P : (k + 1) * P], ident[:]
                        )
                        zqt = zqpool.tile([P, P], F32)
                        nc.scalar.copy(zqt[:], pst[:])
                        nc.sync.dma_start(
                            out=zq_o[
                                bb,
                                k * P : (k + 1) * P,
                                poff + sub * P : poff + (sub + 1) * P,
                            ],
                            in_=zqt[:],
                        )
                    # stash per-row min (as max of negdist) and idx
                    nc.scalar.copy(msb[:, st : st + 1], m8[:, :1])
                    nc.vector.tensor_copy(isb[:, st : st + 1], idx8[:, :1])

            nc.sync.dma_start(out=m_o[:], in_=msb[:])
            nc.sync.dma_start(out=idx_o[:], in_=isb[:])

    _split_waits(nc, limit=1)
    return nc


_NC_CACHE = None


def _get_nc():
    global _NC_CACHE
    if _NC_CACHE is None:
        _NC_CACHE = _build()
    return _NC_CACHE


LAST_RES = None


def kernel(z, codebook, _want_timing=False):
    z = np.ascontiguousarray(z, dtype=np.float32)
    codebook = np.ascontiguousarray(codebook, dtype=np.float32)
    assert z.shape == (B, D, H, W) and codebook.shape == (K, D)

    cbt2 = np.ascontiguousarray((2.0 * codebook).T)            # (512, 1024)
    cn = np.sum(codebook * codebook, axis=1, dtype=np.float32).reshape(1, K)
    zn = np.einsum("bdhw,bdhw->bhw", z, z, dtype=np.float32).astype(np.float32)

    zc_view = z.reshape(B, D, H * W)
    in_maps = []
    for c in range(N_CORES):
        zn_c = zn[c * B_PER_CORE : (c + 1) * B_PER_CORE].reshape(-1)  # (8192,)
        znr = np.ascontiguousarray(zn_c.reshape(N_SUB, P).T)          # (128, 64)
        in_maps.append(
            dict(
                zc=np.ascontiguousarray(zc_view[c * B_PER_CORE : (c + 1) * B_PER_CORE]),
                cbt2=cbt2,
                cn=cn,
                znr=znr,
                cbg=codebook,
            )
        )

    nc = _get_nc()
    res = run_bass_kernel_spmd(nc, in_maps, list(range(N_CORES)), trace=_want_timing)
    global LAST_RES
    LAST_RES = res

    probs = np.concatenate([r["probs"] for r in res.results], axis=0)  # (65536, 1024)
    zq = np.concatenate([r["zq"] for r in res.results], axis=0).reshape(B, D, H, W)

    # q_loss = 1.25 * mean((z_q - z_flat)^2); per-row squared distance equals
    # the fp32 dist at the argmin, which is -mrow.
    tot = 0.0
    for r in res.results:
        tot += -np.sum(r["mrow"].astype(np.float64))
    c_loss = tot / (B * H * W * D)
    q_loss = np.float32(1.25 * c_loss)

    return zq, q_loss, probs
